# revision 26
# baseline (speedup 1.0000x reference)
"""BitSwarmLinear Trainium2 kernel.

Computation (reference):
    swarm_sum = population.sum(axis=2)          # (out, in)
    w_eff     = sign(swarm_sum), sign(0) -> +1  # (out, in), +-1
    y         = einsum("bsi,oi->bso", x, w_eff) # (4, 4096, out)

Distribution (8 NeuronCores, 2D: 2-way tokens x 4-way out_features):
    core c -> token half c//4 (8192 tokens), out quarter c%4 (512 cols).
    Per-core DMA is ~48 MiB (x 32 + pop 8 + y 8) vs the 128x128 PE's
    bf16 floor of ~220us for the 8.6 G-MAC/core matmul -> compute-bound.

Host staging (lossless / layout-only):
    - population is exactly +-1.0 -> one BIT per element, four swarm
      members per byte (2-bit fields), laid out pre-transposed
      [og, in_part, plane, ko, out] so the device reduction lands
      directly in the matmul-lhsT layout W^T[in, out]. 16x less DMA
      than a u8 {0,1} recode and zero on-device transposes.
    - x -> bf16 x^T in 512-token tiles [t2, 128 ki, 16 ko, 512]: 2 MB
      contiguous DMAs (fine-grained startup interleave, line-rate HBM).
    - y returns bf16 [tb, out_part, og, tok] tile-major; host restores
      [b, s, out] f32.

Per-core device pipeline:
    - Startup-critical input DMA (all pop chunks + x tiles 0-7) rides
      the SP (sync) HWDGE ring in an explicit priority order; the bulk
      x tail (x8-x15, 16 MB) is deferred onto the ACT ring and issued
      just-in-time mid-run, keeping the paired NeuronCore's HBM stack
      free during both cores' startup-critical window.
    - W-prep per og block (128 out cols): DVE sums 8 byte-planes into
      three accumulators (2-bit fields stay <= 3: no carries; u16-lane
      packed adds are exact in the fp32 ALU), unpacks the fields with
      shift/mask tensor_scalar ops (all chunk-0-only work first, so a
      late second chunk costs nothing for ~5us), then binarizes in one
      op to {-0.5, +0.5} (count >= 16 <=> sum >= 0, 0 -> +1); the PSUM
      drain's activation scale=2 restores +-1 exactly.
    - Matmuls: per 512-token tile and 128-col out group, 16
      accumulating matmuls (W slice stationary, x moving, N=512) into
      one PSUM bank. (tb, og) groups run in a staggered order (og k
      first needed at group {0, 2, 4, 7}) that opens with a 3-tb x
      window and settles into 2-tb-window rounds so every x-tile pool
      slot frees a full tb (~27us) before its reload is needed.
    - ACT drains PSUM -> bf16 ystage (activation copy, scale=2) and
      issues the 1 MB y stores; DVE is left free for W-prep; the PE
      never waits on a drain (6 PSUM banks rotate).

Measured: ~271-273us HW exec (slowest of 8 cores; PE busy ~221us =
the bf16 roofline for 1024 N=512 matmuls), rel err 2.3e-3 (bf16).
"""

import os
import sys

import numpy as np

for _p in ("/root/.axon_site/_ro/trn_rl_repo", "/opt/trn_rl_repo"):
    if os.path.isdir(_p) and _p not in sys.path:
        sys.path.append(_p)

import ml_dtypes

# bass_utils' axon trace path imports antenv.axon_hooks, which this image
# lacks. Provide it (backed by the ctypes NTFF hook) so running with
# BASS_TRACE=1 works instead of crashing on the import.
try:
    import antenv.axon_hooks  # noqa: F401
except ImportError:
    try:
        import types as _types

        from trn_agent_boot.trn_boot import _ntff_profile_via_ctypes

        _hooks = _types.ModuleType("antenv.axon_hooks")
        _ntff_hook = _ntff_profile_via_ctypes("/opt/axon/libaxon_pjrt.so")
        _hooks.get_axon_ntff_profile_hook = lambda: _ntff_hook
        _hooks.set_axon_ntff_profile_hook = lambda h: None
        sys.modules["antenv.axon_hooks"] = _hooks
    except Exception:
        pass

import concourse.bass as bass  # noqa: F401  (AP helpers)
import concourse.mybir as mybir
import concourse.tile as tile
from concourse import bacc
from concourse.bass_utils import run_bass_kernel_spmd

P = 128
IN_F = 2048
SWARM = 32
OUT_F = 2048
N_CORES = 8
TOK_WAYS = 2
OUT_WAYS = 4
TOKENS = 4 * 4096
TOK_C = TOKENS // TOK_WAYS      # 8192 tokens per core
OUT_C = OUT_F // OUT_WAYS       # 512 out features per core
KO = IN_F // P                  # 16 K-tiles
OG = OUT_C // P                 # 4 out groups of 128
PLANES = SWARM // 4             # 8 byte-planes (4 swarm bits per byte)
T2 = 512                        # tokens per x tile / PSUM group
TB = 1024                       # tokens per y store
TH = TB // T2                   # 2 PSUM groups per y store

F32 = mybir.dt.float32
BF16 = mybir.dt.bfloat16
U8 = mybir.dt.uint8
U16 = mybir.dt.uint16

M2 = 0x3333  # 2-bit field pairs (bits 0-1, 4-5 of each byte)
M4 = 0x0F0F  # low nibble of both bytes in a u16 lane
ALU = mybir.AluOpType

XT_BUFS = 8   # 512-token tiles: 3-tb window + 2 prefetch slots
PT_BUFS = 2
YS_BUFS = 4

# staggered (tb, og) matmul order for tb_count == 8: og k first needed at
# group index {0:0, 1:2, 2:4, 3:7} (W-prep pipeline slack), 3-tb x window
# at the start, then 2-tb-window rounds [(r,0),(r-1,1..3)] so each x-tile
# slot frees a full tb (~27us) before its reload is needed (prefetch
# margin; a pure 3-tb window gives the pool ZERO lead and starves the PE)
_MM_ORDER_8 = [
    (0, 0),
    (1, 0), (0, 1), (1, 1), (2, 0),
    (0, 2), (1, 2), (0, 3), (1, 3),
    (3, 0), (2, 1), (2, 2), (2, 3),
    (4, 0), (3, 1), (3, 2), (3, 3),
    (5, 0), (4, 1), (4, 2), (4, 3),
    (6, 0), (5, 1), (5, 2), (5, 3),
    (7, 0), (6, 1), (6, 2), (6, 3),
    (7, 1), (7, 2), (7, 3),
]

# input DMA priority order on the sync ring: ("pop", og, chunk) is a 1 MB
# 4-plane chunk, ("x", t2) is a 2 MB x tile. og0 first (gates first MM),
# then x/og interleaved to meet the staggered schedule's deadlines.
_LOAD_ORDER_8 = [
    ("pop", 0, 0), ("pop", 0, 1), ("pop", 1, 0), ("pop", 1, 1),
    ("x", 0), ("x", 1), ("x", 2), ("x", 3),
    ("pop", 2, 0), ("pop", 2, 1),
    ("pop", 3, 0), ("pop", 3, 1),
] + [("x", t) for t in range(4, 8)]

# bulk x tiles (x8-x15) are deferred to the ACT ring, issued just-in-time
# after these matmul group indices: keeps the 16 MB of tail prefetch out
# of the 0-60us window where the paired NeuronCore (same HBM stack) is
# streaming its own startup-critical bytes. ~4 group-slots (~27us) of
# lead each; the slot wait is always satisfied before issue, so the ACT
# ring never blocks.
_LATE_X_8 = {9: (8, 9), 13: (10, 11), 17: (12, 13), 21: (14, 15)}


def build_nc(tb_count: int = TOK_C // TB):
    """Build the per-core Bass program (same program on all 8 cores)."""
    t2_count = tb_count * TH
    if tb_count == TOK_C // TB:
        mm_order = _MM_ORDER_8
        load_order = _LOAD_ORDER_8
    else:
        mm_order = [(t, og) for t in range(tb_count) for og in range(OG)]
        load_order = (
            [("pop", og, pc) for og in range(OG) for pc in range(2)]
            + [("x", t) for t in range(t2_count)]
        )

    nc = bacc.Bacc(
        "TRN2",
        target_bir_lowering=False,
        debug=False,
        enable_asserts=False,
        num_devices=N_CORES,
    )

    xT = nc.dram_tensor("xT", [t2_count, P, KO, T2], BF16,
                        kind="ExternalInput")
    # 2-bit-packed population: [og, in_part, plane, ko, out128]
    nib = nc.dram_tensor("nib", [OG, P, PLANES, KO, P], U8,
                         kind="ExternalInput")
    y = nc.dram_tensor("y", [tb_count, P, OG, TB], BF16,
                       kind="ExternalOutput")

    xr = xT.ap()
    nr = nib.ap()
    yr = y.ap()

    with tile.TileContext(nc) as tc:
        with (
            tc.tile_pool(name="wsb", bufs=1) as w_pool,
            tc.tile_pool(name="pt", bufs=PT_BUFS) as pt_pool,
            tc.tile_pool(name="acc", bufs=1) as acc_pool,
            tc.tile_pool(name="tmp", bufs=1) as tmp_pool,
            tc.tile_pool(name="xt", bufs=XT_BUFS) as x_pool,
            tc.tile_pool(name="ys", bufs=YS_BUFS) as y_pool,
            tc.tile_pool(name="psum_y", bufs=6, space="PSUM") as psum_pool,
        ):
            # W^T [in_part, ko, out] bf16 -- matmul lhsT slices, SBUF-resident
            w_sb = w_pool.tile([P, KO, OUT_C], BF16, tag="wsb")

            # ---- emit load DMAs (sync ring, priority order) + W-prep (DVE)
            xt_tiles: dict = {}
            pop_chunks: dict = {}
            for item in load_order:
                if item[0] == "x":
                    t = item[1]
                    xt = x_pool.tile([P, KO, T2], BF16, tag="xt", name=f"xt{t}")
                    nc.sync.dma_start(xt[:], xr[t])
                    xt_tiles[t] = xt
                else:
                    _, og, pc = item
                    pt = pt_pool.tile([P, 4, KO, P], U8, tag="pt", name=f"pt{og}_{pc}")
                    nc.sync.dma_start(pt[:], nr[og, :, 4 * pc : 4 * pc + 4])
                    pop_chunks[(og, pc)] = pt
                    if pc == 1:
                        _emit_wprep(nc, w_sb, pop_chunks, acc_pool, tmp_pool,
                                    og)

            # ---- matmuls (PE) + drains and y stores (ACT)
            late_x = _LATE_X_8 if tb_count == TOK_C // TB else {}
            ystages: dict = {}
            done_ogs: dict = {}
            for gi, (tb, og) in enumerate(mm_order):
                for t in late_x.get(gi, ()):
                    xt = x_pool.tile([P, KO, T2], BF16, tag="xt",
                                     name=f"xt{t}")
                    nc.scalar.dma_start(xt[:], xr[t])
                    xt_tiles[t] = xt
                if tb not in ystages:
                    ystages[tb] = y_pool.tile([P, OG, TB], BF16, tag="ys", name=f"ys{tb}")
                    done_ogs[tb] = 0
                for th in range(TH):
                    xt = xt_tiles[tb * TH + th]
                    ps = psum_pool.tile([P, T2], F32, tag="yps", name=f"ps{tb}_{og}_{th}")
                    for k in range(KO):
                        nc.tensor.matmul(
                            ps[:],
                            w_sb[:, k, og * P : (og + 1) * P],
                            xt[:, k, :],
                            start=(k == 0),
                            stop=(k == KO - 1),
                        )
                    nc.scalar.mul(
                        out=ystages[tb][:, og, th * T2 : (th + 1) * T2],
                        in_=ps[:],
                        mul=2.0,
                    )
                done_ogs[tb] += 1
                if done_ogs[tb] == OG:
                    nc.scalar.dma_start(yr[tb], ystages[tb][:])

    nc.compile()
    return nc


def _emit_wprep(nc, w_sb, pop_chunks, acc_pool, tmp_pool, og):
    """DVE: sum 8 planes (2-bit fields) -> counts -> +-1 bf16 W og-slice."""
    c0 = pop_chunks[(og, 0)][:].bitcast(U16)  # [128, 4, KO, 64] planes 0-3
    c1 = pop_chunks[(og, 1)][:].bitcast(U16)  # planes 4-7
    acc = acc_pool.tile([P, 3, KO, P // 2], U16, tag="acc", name=f"acc{og}")
    a, b, d = acc[:, 0], acc[:, 1], acc[:, 2]

    u = tmp_pool.tile([P, KO, P // 2], U16, tag="u", name=f"u{og}")
    t = tmp_pool.tile([P, KO, P // 2], U16, tag="t", name=f"t{og}")
    v = tmp_pool.tile([P, KO, P // 2], U16, tag="v", name=f"v{og}")
    cnt = tmp_pool.tile([P, KO, P // 2], U16, tag="cnt", name=f"cnt{og}")

    def _unpack(src, into_cnt):
        # (src & M2) + ((src >> 2) & M2) -> nibble fields <= 6, then
        # (v & M4) + ((v >> 4) & M4) -> byte counts <= 12. (The BIR
        # verifier forbids mixing bitwise op0 with arith op1, so mask
        # and add stay separate instructions.)
        nc.vector.tensor_scalar(
            out=u[:], in0=src, scalar1=2, scalar2=M2,
            op0=ALU.logical_shift_right, op1=ALU.bitwise_and,
        )
        nc.vector.tensor_scalar(
            out=t[:], in0=src, scalar1=M2, scalar2=None,
            op0=ALU.bitwise_and,
        )
        nc.vector.tensor_add(v[:], t[:], u[:])
        nc.vector.tensor_scalar(
            out=u[:], in0=v[:], scalar1=4, scalar2=M4,
            op0=ALU.logical_shift_right, op1=ALU.bitwise_and,
        )
        nc.vector.tensor_scalar(
            out=t[:], in0=v[:], scalar1=M4, scalar2=None,
            op0=ALU.bitwise_and,
        )
        if into_cnt:
            nc.vector.tensor_add(cnt[:], t[:], u[:])
        else:
            nc.vector.tensor_add(v[:], t[:], u[:])
            nc.vector.tensor_add(cnt[:], cnt[:], v[:])

    # three accumulators keep every 2-bit field <= 3 (no carries); u16
    # lane values stay <= 0xFFFF (exact in the fp32 ALU). All chunk-0
    # work (a + its unpack) is emitted before anything touching chunk 1
    # so a late second chunk costs nothing until ~5us into the og.
    nc.vector.tensor_add(a, c0[:, 0], c0[:, 1])
    nc.vector.tensor_add(a, a, c0[:, 2])
    _unpack(a, True)
    nc.vector.tensor_add(b, c0[:, 3], c1[:, 0])
    nc.vector.tensor_add(b, b, c1[:, 1])
    nc.vector.tensor_add(d, c1[:, 2], c1[:, 3])
    _unpack(b, False)
    _unpack(d, False)

    cnt8 = cnt[:].bitcast(U8)  # [128, KO, 128] counts in [0, 32]
    wslice = w_sb[:, :, og * P : (og + 1) * P]
    # count >= 16  <=>  swarm_sum >= 0. One op: w = (count >= 16) - 0.5
    # in {-0.5, +0.5}; the PSUM drain's activation scale=2 restores +-1
    # (exact: power-of-2 scale on an fp32 accumulation).
    nc.vector.tensor_scalar(
        out=wslice, in0=cnt8, scalar1=16, scalar2=0.5,
        op0=ALU.is_ge, op1=ALU.subtract,
    )


_NC_CACHE: dict = {}


def _get_nc(tb_count: int = TOK_C // TB):
    if tb_count not in _NC_CACHE:
        _NC_CACHE[tb_count] = build_nc(tb_count)
    return _NC_CACHE[tb_count]


def stage_x(x: np.ndarray):
    """x [b, s, in] f32 -> tiled bf16 x^T [t2_total, 128 ki, ko, 512]."""
    tokens = x.shape[0] * x.shape[1]
    xb = np.ascontiguousarray(
        x.reshape(tokens, IN_F).T
    ).astype(ml_dtypes.bfloat16)  # [in, tokens]
    t2t = tokens // T2
    # (ko ki) (t2 t) -> t2 ki ko t
    return np.ascontiguousarray(
        xb.reshape(KO, P, t2t, T2).transpose(2, 1, 0, 3)
    )


def stage_pop_quarter(pop_q: np.ndarray):
    """pop slice [512 out, in, 32] (+-1.0 f32) -> 2-bit-packed u8
    [og, ki, plane, ko, out128]; byte(plane p) = sum_j bit(s=8j+p) << 2j.
    Lossless layout-only recode (one bit per population element)."""
    b = (pop_q > 0).astype(np.uint8)  # [out 512, in 2048, s 32]
    b = b.reshape(OUT_C, IN_F, 4, PLANES)  # [out, in, j, p]
    byt = (
        b[:, :, 0, :] | (b[:, :, 1, :] << 2)
        | (b[:, :, 2, :] << 4) | (b[:, :, 3, :] << 6)
    )  # [out, in, p]
    byt = byt.reshape(OG, P, KO, P, PLANES)  # [og, o, ko, ki, p]
    return np.ascontiguousarray(byt.transpose(0, 3, 4, 2, 1))


def unstage_y(y_dev: np.ndarray):
    """y [tb, 128 r, og, TB t] bf16 -> [tok_c, out_c] f32
    (token = tb*TB + t, out = og*128 + r)."""
    tbc = y_dev.shape[0]
    return (
        y_dev.astype(np.float32)
        .transpose(0, 3, 2, 1)
        .reshape(tbc * TB, OUT_C)
    )


def prep_inputs(x: np.ndarray, population: np.ndarray):
    xT = stage_x(x)
    t2_half = TOK_C // T2
    nib_q = [
        stage_pop_quarter(population[q * OUT_C : (q + 1) * OUT_C])
        for q in range(OUT_WAYS)
    ]
    in_maps = []
    for c in range(N_CORES):
        h, q = c // OUT_WAYS, c % OUT_WAYS
        in_maps.append({
            "xT": xT[h * t2_half : (h + 1) * t2_half],
            "nib": nib_q[q],
        })
    return in_maps


def gather_y(results, batch_shape):
    y_full = np.empty((TOKENS, OUT_F), dtype=np.float32)
    for c, r in enumerate(results):
        h, q = c // OUT_WAYS, c % OUT_WAYS
        y_full[h * TOK_C : (h + 1) * TOK_C, q * OUT_C : (q + 1) * OUT_C] = (
            unstage_y(r["y"])
        )
    return y_full.reshape(*batch_shape, OUT_F)


def kernel(x: np.ndarray, population: np.ndarray):
    in_maps = prep_inputs(x, population)
    nc = _get_nc()
    res = run_bass_kernel_spmd(nc, in_maps, core_ids=list(range(N_CORES)))
    return gather_y(res.results, x.shape[:2])


# revision 27
# speedup vs baseline: 1.1320x; 1.1320x over previous
"""BitSwarmLinear Trainium2 kernel.

Computation (reference):
    swarm_sum = population.sum(axis=2)          # (out, in)
    w_eff     = sign(swarm_sum), sign(0) -> +1  # (out, in), +-1
    y         = einsum("bsi,oi->bso", x, w_eff) # (4, 4096, out)

Distribution (8 NeuronCores, 2D: 2-way tokens x 4-way out_features):
    core c -> token half c//4 (8192 tokens), out quarter c%4 (512 cols).
    Per-core DMA is ~48 MiB (x 32 + pop 8 + y 8) vs the 128x128 PE's
    bf16 floor of ~220us for the 8.6 G-MAC/core matmul -> compute-bound.

Host staging (lossless / layout-only):
    - population is exactly +-1.0 -> one BIT per element, four swarm
      members per byte (2-bit fields), laid out pre-transposed
      [og, in_part, plane, ko, out] so the device reduction lands
      directly in the matmul-lhsT layout W^T[in, out]. 16x less DMA
      than a u8 {0,1} recode and zero on-device transposes.
    - x -> bf16 x^T in 512-token tiles [t2, 128 ki, 16 ko, 512]: 2 MB
      contiguous DMAs (fine-grained startup interleave, line-rate HBM).
    - y returns bf16 [tb, out_part, og, tok] tile-major; host restores
      [b, s, out] f32.

Per-core device pipeline:
    - Startup-critical input DMA (all pop chunks + x tiles 0-7) rides
      the SP (sync) HWDGE ring in an explicit priority order; the bulk
      x tail (x8-x15, 16 MB) is deferred onto the ACT ring and issued
      just-in-time mid-run, keeping the paired NeuronCore's HBM stack
      free during both cores' startup-critical window.
    - W-prep per og block (128 out cols): DVE sums 8 byte-planes into
      three accumulators (2-bit fields stay <= 3: no carries; u16-lane
      packed adds are exact in the fp32 ALU), unpacks the fields with
      shift/mask tensor_scalar ops (all chunk-0-only work first, so a
      late second chunk costs nothing for ~5us), then binarizes in one
      op to {-0.5, +0.5} (count >= 16 <=> sum >= 0, 0 -> +1); the PSUM
      drain's activation scale=2 restores +-1 exactly.
    - Matmuls: per 512-token tile and 128-col out group, 16
      accumulating matmuls (W slice stationary, x moving, N=512) into
      one PSUM bank. (tb, og) groups run in a staggered order (og k
      first needed at group {0, 2, 4, 7}) that opens with a 3-tb x
      window and settles into 2-tb-window rounds so every x-tile pool
      slot frees a full tb (~27us) before its reload is needed.
    - ACT drains PSUM -> bf16 ystage (activation copy, scale=2) and
      issues the 1 MB y stores; DVE is left free for W-prep; the PE
      never waits on a drain (6 PSUM banks rotate).

Measured: ~271-273us HW exec (slowest of 8 cores; PE busy ~221us =
the bf16 roofline for 1024 N=512 matmuls), rel err 2.3e-3 (bf16).
"""

import os
import sys

import numpy as np

for _p in ("/root/.axon_site/_ro/trn_rl_repo", "/opt/trn_rl_repo"):
    if os.path.isdir(_p) and _p not in sys.path:
        sys.path.append(_p)

import ml_dtypes

# bass_utils' axon trace path imports antenv.axon_hooks, which this image
# lacks. Provide it (backed by the ctypes NTFF hook) so running with
# BASS_TRACE=1 works instead of crashing on the import.
try:
    import antenv.axon_hooks  # noqa: F401
except ImportError:
    try:
        import types as _types

        from trn_agent_boot.trn_boot import _ntff_profile_via_ctypes

        _hooks = _types.ModuleType("antenv.axon_hooks")
        _ntff_hook = _ntff_profile_via_ctypes("/opt/axon/libaxon_pjrt.so")
        _hooks.get_axon_ntff_profile_hook = lambda: _ntff_hook
        _hooks.set_axon_ntff_profile_hook = lambda h: None
        sys.modules["antenv.axon_hooks"] = _hooks
    except Exception:
        pass

import concourse.bass as bass  # noqa: F401  (AP helpers)
import concourse.mybir as mybir
import concourse.tile as tile
from concourse import bacc
from concourse.bass_utils import run_bass_kernel_spmd

P = 128
IN_F = 2048
SWARM = 32
OUT_F = 2048
N_CORES = 8
TOK_WAYS = 2
OUT_WAYS = 4
TOKENS = 4 * 4096
TOK_C = TOKENS // TOK_WAYS      # 8192 tokens per core
OUT_C = OUT_F // OUT_WAYS       # 512 out features per core
KO = IN_F // P                  # 16 K-tiles
OG = OUT_C // P                 # 4 out groups of 128
PLANES = SWARM // 4             # 8 byte-planes (4 swarm bits per byte)
T2 = 512                        # tokens per x tile / PSUM group
TB = 1024                       # tokens per y store
TH = TB // T2                   # 2 PSUM groups per y store

F32 = mybir.dt.float32
BF16 = mybir.dt.bfloat16
U8 = mybir.dt.uint8
U16 = mybir.dt.uint16

M2 = 0x3333  # 2-bit field pairs (bits 0-1, 4-5 of each byte)
M4 = 0x0F0F  # low nibble of both bytes in a u16 lane
ALU = mybir.AluOpType

XT_BUFS = 7   # 512-token tiles: 3-tb window + 1 prefetch slot
PT_BUFS = 4
YS_BUFS = 4

# staggered (tb, og) matmul order for tb_count == 8: og k first needed at
# group index {0:0, 1:2, 2:4, 3:7} (W-prep pipeline slack), 3-tb x window
# at the start, then 2-tb-window rounds [(r,0),(r-1,1..3)] so each x-tile
# slot frees a full tb (~27us) before its reload is needed (prefetch
# margin; a pure 3-tb window gives the pool ZERO lead and starves the PE)
_MM_ORDER_8 = [
    (0, 0),
    (1, 0), (0, 1), (1, 1), (2, 0),
    (0, 2), (1, 2), (0, 3), (1, 3),
    (3, 0), (2, 1), (2, 2), (2, 3),
    (4, 0), (3, 1), (3, 2), (3, 3),
    (5, 0), (4, 1), (4, 2), (4, 3),
    (6, 0), (5, 1), (5, 2), (5, 3),
    (7, 0), (6, 1), (6, 2), (6, 3),
    (7, 1), (7, 2), (7, 3),
]

# input DMA priority order on the sync ring: ("pop", og, chunk) is a 1 MB
# 4-plane chunk, ("x", t2) is a 2 MB x tile. og0 first (gates first MM),
# then x/og interleaved to meet the staggered schedule's deadlines.
_LOAD_ORDER_8 = [
    ("pop", 0, 0), ("pop", 0, 1), ("pop", 1, 0), ("pop", 1, 1),
    ("x", 0), ("x", 1), ("x", 2),
    ("pop", 2, 0), ("pop", 2, 1), ("x", 3),
    ("pop", 3, 0), ("pop", 3, 1),
] + [("x", t) for t in range(4, 8)]

# bulk x tiles (x8-x15) are deferred to the ACT ring, issued just-in-time
# after these matmul group indices: keeps the 16 MB of tail prefetch out
# of the 0-60us window where the paired NeuronCore (same HBM stack) is
# streaming its own startup-critical bytes. ~4 group-slots (~27us) of
# lead each; the slot wait is always satisfied before issue, so the ACT
# ring never blocks.
_LATE_X_8 = {9: (8, 9), 13: (10, 11), 17: (12, 13), 21: (14, 15)}


def build_nc(tb_count: int = TOK_C // TB):
    """Build the per-core Bass program (same program on all 8 cores)."""
    t2_count = tb_count * TH
    if tb_count == TOK_C // TB:
        mm_order = _MM_ORDER_8
        load_order = _LOAD_ORDER_8
    else:
        mm_order = [(t, og) for t in range(tb_count) for og in range(OG)]
        load_order = (
            [("pop", og, pc) for og in range(OG) for pc in range(2)]
            + [("x", t) for t in range(t2_count)]
        )

    nc = bacc.Bacc(
        "TRN2",
        target_bir_lowering=False,
        debug=False,
        enable_asserts=False,
        num_devices=N_CORES,
    )

    xT = nc.dram_tensor("xT", [t2_count, P, KO, T2], BF16,
                        kind="ExternalInput")
    # 2-bit-packed population: [og, in_part, plane, ko, out128]
    nib = nc.dram_tensor("nib", [OG, P, PLANES, KO, P], U8,
                         kind="ExternalInput")
    y = nc.dram_tensor("y", [tb_count, P, OG, TB], BF16,
                       kind="ExternalOutput")

    xr = xT.ap()
    nr = nib.ap()
    yr = y.ap()

    with tile.TileContext(nc) as tc:
        with (
            tc.tile_pool(name="wsb", bufs=1) as w_pool,
            tc.tile_pool(name="pt", bufs=PT_BUFS) as pt_pool,
            tc.tile_pool(name="acc", bufs=1) as acc_pool,
            tc.tile_pool(name="tmp", bufs=1) as tmp_pool,
            tc.tile_pool(name="xt", bufs=XT_BUFS) as x_pool,
            tc.tile_pool(name="ys", bufs=YS_BUFS) as y_pool,
            tc.tile_pool(name="psum_y", bufs=6, space="PSUM") as psum_pool,
        ):
            # W^T [in_part, ko, out] bf16 -- matmul lhsT slices, SBUF-resident
            w_sb = w_pool.tile([P, KO, OUT_C], BF16, tag="wsb")

            # ---- emit load DMAs (sync ring, priority order) + W-prep (DVE)
            xt_tiles: dict = {}
            pop_chunks: dict = {}
            for item in load_order:
                if item[0] == "x":
                    t = item[1]
                    xt = x_pool.tile([P, KO, T2], BF16, tag="xt", name=f"xt{t}")
                    nc.sync.dma_start(xt[:], xr[t])
                    xt_tiles[t] = xt
                else:
                    _, og, pc = item
                    pt = pt_pool.tile([P, 4, KO, P], U8, tag="pt", name=f"pt{og}_{pc}")
                    nc.sync.dma_start(pt[:], nr[og, :, 4 * pc : 4 * pc + 4])
                    pop_chunks[(og, pc)] = pt
                    if pc == 1:
                        _emit_wprep(nc, w_sb, pop_chunks, acc_pool, tmp_pool,
                                    og)

            # ---- matmuls (PE) + drains and y stores (ACT)
            late_x = _LATE_X_8 if tb_count == TOK_C // TB else {}
            ystages: dict = {}
            done_ogs: dict = {}
            for gi, (tb, og) in enumerate(mm_order):
                for t in late_x.get(gi, ()):
                    xt = x_pool.tile([P, KO, T2], BF16, tag="xt",
                                     name=f"xt{t}")
                    nc.scalar.dma_start(xt[:], xr[t])
                    xt_tiles[t] = xt
                if tb not in ystages:
                    ystages[tb] = y_pool.tile([P, OG, TB], BF16, tag="ys", name=f"ys{tb}")
                    done_ogs[tb] = 0
                for th in range(TH):
                    xt = xt_tiles[tb * TH + th]
                    ps = psum_pool.tile([P, T2], F32, tag="yps", name=f"ps{tb}_{og}_{th}")
                    for k in range(KO):
                        nc.tensor.matmul(
                            ps[:],
                            w_sb[:, k, og * P : (og + 1) * P],
                            xt[:, k, :],
                            start=(k == 0),
                            stop=(k == KO - 1),
                        )
                    nc.scalar.mul(
                        out=ystages[tb][:, og, th * T2 : (th + 1) * T2],
                        in_=ps[:],
                        mul=2.0,
                    )
                done_ogs[tb] += 1
                if done_ogs[tb] == OG:
                    nc.scalar.dma_start(yr[tb], ystages[tb][:])

    nc.compile()
    return nc


def _emit_wprep(nc, w_sb, pop_chunks, acc_pool, tmp_pool, og):
    """DVE: sum 8 planes (2-bit fields) -> counts -> +-1 bf16 W og-slice."""
    c0 = pop_chunks[(og, 0)][:].bitcast(U16)  # [128, 4, KO, 64] planes 0-3
    c1 = pop_chunks[(og, 1)][:].bitcast(U16)  # planes 4-7
    acc = acc_pool.tile([P, 3, KO, P // 2], U16, tag="acc", name=f"acc{og}")
    a, b, d = acc[:, 0], acc[:, 1], acc[:, 2]

    u = tmp_pool.tile([P, KO, P // 2], U16, tag="u", name=f"u{og}")
    t = tmp_pool.tile([P, KO, P // 2], U16, tag="t", name=f"t{og}")
    v = tmp_pool.tile([P, KO, P // 2], U16, tag="v", name=f"v{og}")
    cnt = tmp_pool.tile([P, KO, P // 2], U16, tag="cnt", name=f"cnt{og}")

    def _unpack(src, into_cnt):
        # (src & M2) + ((src >> 2) & M2) -> nibble fields <= 6, then
        # (v & M4) + ((v >> 4) & M4) -> byte counts <= 12. (The BIR
        # verifier forbids mixing bitwise op0 with arith op1, so mask
        # and add stay separate instructions.)
        nc.vector.tensor_scalar(
            out=u[:], in0=src, scalar1=2, scalar2=M2,
            op0=ALU.logical_shift_right, op1=ALU.bitwise_and,
        )
        nc.vector.tensor_scalar(
            out=t[:], in0=src, scalar1=M2, scalar2=None,
            op0=ALU.bitwise_and,
        )
        nc.vector.tensor_add(v[:], t[:], u[:])
        nc.vector.tensor_scalar(
            out=u[:], in0=v[:], scalar1=4, scalar2=M4,
            op0=ALU.logical_shift_right, op1=ALU.bitwise_and,
        )
        nc.vector.tensor_scalar(
            out=t[:], in0=v[:], scalar1=M4, scalar2=None,
            op0=ALU.bitwise_and,
        )
        if into_cnt:
            nc.vector.tensor_add(cnt[:], t[:], u[:])
        else:
            nc.vector.tensor_add(v[:], t[:], u[:])
            nc.vector.tensor_add(cnt[:], cnt[:], v[:])

    # three accumulators keep every 2-bit field <= 3 (no carries); u16
    # lane values stay <= 0xFFFF (exact in the fp32 ALU). All chunk-0
    # work (a + its unpack) is emitted before anything touching chunk 1
    # so a late second chunk costs nothing until ~5us into the og.
    nc.vector.tensor_add(a, c0[:, 0], c0[:, 1])
    nc.vector.tensor_add(a, a, c0[:, 2])
    _unpack(a, True)
    nc.vector.tensor_add(b, c0[:, 3], c1[:, 0])
    nc.vector.tensor_add(b, b, c1[:, 1])
    nc.vector.tensor_add(d, c1[:, 2], c1[:, 3])
    _unpack(b, False)
    _unpack(d, False)

    cnt8 = cnt[:].bitcast(U8)  # [128, KO, 128] counts in [0, 32]
    wslice = w_sb[:, :, og * P : (og + 1) * P]
    # count >= 16  <=>  swarm_sum >= 0. One op: w = (count >= 16) - 0.5
    # in {-0.5, +0.5}; the PSUM drain's activation scale=2 restores +-1
    # (exact: power-of-2 scale on an fp32 accumulation).
    nc.vector.tensor_scalar(
        out=wslice, in0=cnt8, scalar1=16, scalar2=0.5,
        op0=ALU.is_ge, op1=ALU.subtract,
    )


_NC_CACHE: dict = {}


def _get_nc(tb_count: int = TOK_C // TB):
    if tb_count not in _NC_CACHE:
        _NC_CACHE[tb_count] = build_nc(tb_count)
    return _NC_CACHE[tb_count]


def stage_x(x: np.ndarray):
    """x [b, s, in] f32 -> tiled bf16 x^T [t2_total, 128 ki, ko, 512]."""
    tokens = x.shape[0] * x.shape[1]
    xb = np.ascontiguousarray(
        x.reshape(tokens, IN_F).T
    ).astype(ml_dtypes.bfloat16)  # [in, tokens]
    t2t = tokens // T2
    # (ko ki) (t2 t) -> t2 ki ko t
    return np.ascontiguousarray(
        xb.reshape(KO, P, t2t, T2).transpose(2, 1, 0, 3)
    )


def stage_pop_quarter(pop_q: np.ndarray):
    """pop slice [512 out, in, 32] (+-1.0 f32) -> 2-bit-packed u8
    [og, ki, plane, ko, out128]; byte(plane p) = sum_j bit(s=8j+p) << 2j.
    Lossless layout-only recode (one bit per population element)."""
    b = (pop_q > 0).astype(np.uint8)  # [out 512, in 2048, s 32]
    b = b.reshape(OUT_C, IN_F, 4, PLANES)  # [out, in, j, p]
    byt = (
        b[:, :, 0, :] | (b[:, :, 1, :] << 2)
        | (b[:, :, 2, :] << 4) | (b[:, :, 3, :] << 6)
    )  # [out, in, p]
    byt = byt.reshape(OG, P, KO, P, PLANES)  # [og, o, ko, ki, p]
    return np.ascontiguousarray(byt.transpose(0, 3, 4, 2, 1))


def unstage_y(y_dev: np.ndarray):
    """y [tb, 128 r, og, TB t] bf16 -> [tok_c, out_c] f32
    (token = tb*TB + t, out = og*128 + r)."""
    tbc = y_dev.shape[0]
    return (
        y_dev.astype(np.float32)
        .transpose(0, 3, 2, 1)
        .reshape(tbc * TB, OUT_C)
    )


def prep_inputs(x: np.ndarray, population: np.ndarray):
    xT = stage_x(x)
    t2_half = TOK_C // T2
    nib_q = [
        stage_pop_quarter(population[q * OUT_C : (q + 1) * OUT_C])
        for q in range(OUT_WAYS)
    ]
    in_maps = []
    for c in range(N_CORES):
        h, q = c // OUT_WAYS, c % OUT_WAYS
        in_maps.append({
            "xT": xT[h * t2_half : (h + 1) * t2_half],
            "nib": nib_q[q],
        })
    return in_maps


def gather_y(results, batch_shape):
    y_full = np.empty((TOKENS, OUT_F), dtype=np.float32)
    for c, r in enumerate(results):
        h, q = c // OUT_WAYS, c % OUT_WAYS
        y_full[h * TOK_C : (h + 1) * TOK_C, q * OUT_C : (q + 1) * OUT_C] = (
            unstage_y(r["y"])
        )
    return y_full.reshape(*batch_shape, OUT_F)


def kernel(x: np.ndarray, population: np.ndarray):
    in_maps = prep_inputs(x, population)
    nc = _get_nc()
    res = run_bass_kernel_spmd(nc, in_maps, core_ids=list(range(N_CORES)))
    return gather_y(res.results, x.shape[:2])


# revision 28
# speedup vs baseline: 1.1501x; 1.0160x over previous
"""BitSwarmLinear Trainium2 kernel.

Computation (reference):
    swarm_sum = population.sum(axis=2)          # (out, in)
    w_eff     = sign(swarm_sum), sign(0) -> +1  # (out, in), +-1
    y         = einsum("bsi,oi->bso", x, w_eff) # (4, 4096, out)

Distribution (8 NeuronCores, 2D: 2-way tokens x 4-way out_features):
    core c -> token half c//4 (8192 tokens), out quarter c%4 (512 cols).
    Per-core DMA is ~48 MiB (x 32 + pop 8 + y 8) vs the 128x128 PE's
    bf16 floor of ~220us for the 8.6 G-MAC/core matmul -> compute-bound.

Host staging (lossless / layout-only):
    - population is exactly +-1.0 -> one BIT per element, four swarm
      members per byte (2-bit fields), laid out pre-transposed
      [og, in_part, plane, ko, out] so the device reduction lands
      directly in the matmul-lhsT layout W^T[in, out]. 16x less DMA
      than a u8 {0,1} recode and zero on-device transposes.
    - x -> bf16 x^T in 512-token tiles [t2, 128 ki, 16 ko, 512]: 2 MB
      contiguous DMAs (fine-grained startup interleave, line-rate HBM).
    - y returns bf16 [tb, out_part, og, tok] tile-major; host restores
      [b, s, out] f32.

Per-core device pipeline:
    - Startup-critical input DMA (all pop chunks + x tiles 0-7) rides
      the SP (sync) HWDGE ring in an explicit priority order; the bulk
      x tail (x8-x15, 16 MB) is deferred onto the ACT ring and issued
      just-in-time mid-run, keeping the paired NeuronCore's HBM stack
      free during both cores' startup-critical window.
    - W-prep per og block (128 out cols): DVE sums 8 byte-planes into
      three accumulators (2-bit fields stay <= 3: no carries; u16-lane
      packed adds are exact in the fp32 ALU), unpacks the fields with
      shift/mask tensor_scalar ops (all chunk-0-only work first, so a
      late second chunk costs nothing for ~5us), then binarizes in one
      op to {-0.5, +0.5} (count >= 16 <=> sum >= 0, 0 -> +1); the PSUM
      drain's activation scale=2 restores +-1 exactly.
    - Matmuls: per 512-token tile and 128-col out group, 16
      accumulating matmuls (W slice stationary, x moving, N=512) into
      one PSUM bank. (tb, og) groups run in a staggered order (og k
      first needed at group {0, 2, 4, 7}) that opens with a 3-tb x
      window and settles into 2-tb-window rounds so every x-tile pool
      slot frees a full tb (~27us) before its reload is needed.
    - ACT drains PSUM -> bf16 ystage (activation copy, scale=2) and
      issues the 1 MB y stores; DVE is left free for W-prep; the PE
      never waits on a drain (6 PSUM banks rotate).

Measured: ~271-273us HW exec (slowest of 8 cores; PE busy ~221us =
the bf16 roofline for 1024 N=512 matmuls), rel err 2.3e-3 (bf16).
"""

import os
import sys

import numpy as np

for _p in ("/root/.axon_site/_ro/trn_rl_repo", "/opt/trn_rl_repo"):
    if os.path.isdir(_p) and _p not in sys.path:
        sys.path.append(_p)

import ml_dtypes

# bass_utils' axon trace path imports antenv.axon_hooks, which this image
# lacks. Provide it (backed by the ctypes NTFF hook) so running with
# BASS_TRACE=1 works instead of crashing on the import.
try:
    import antenv.axon_hooks  # noqa: F401
except ImportError:
    try:
        import types as _types

        from trn_agent_boot.trn_boot import _ntff_profile_via_ctypes

        _hooks = _types.ModuleType("antenv.axon_hooks")
        _ntff_hook = _ntff_profile_via_ctypes("/opt/axon/libaxon_pjrt.so")
        _hooks.get_axon_ntff_profile_hook = lambda: _ntff_hook
        _hooks.set_axon_ntff_profile_hook = lambda h: None
        sys.modules["antenv.axon_hooks"] = _hooks
    except Exception:
        pass

import concourse.bass as bass  # noqa: F401  (AP helpers)
import concourse.mybir as mybir
import concourse.tile as tile
from concourse import bacc
from concourse.bass_utils import run_bass_kernel_spmd

P = 128
IN_F = 2048
SWARM = 32
OUT_F = 2048
N_CORES = 8
TOK_WAYS = 2
OUT_WAYS = 4
TOKENS = 4 * 4096
TOK_C = TOKENS // TOK_WAYS      # 8192 tokens per core
OUT_C = OUT_F // OUT_WAYS       # 512 out features per core
KO = IN_F // P                  # 16 K-tiles
OG = OUT_C // P                 # 4 out groups of 128
PLANES = SWARM // 4             # 8 byte-planes (4 swarm bits per byte)
T2 = 512                        # tokens per x tile / PSUM group
TB = 1024                       # tokens per y store
TH = TB // T2                   # 2 PSUM groups per y store

F32 = mybir.dt.float32
BF16 = mybir.dt.bfloat16
U8 = mybir.dt.uint8
U16 = mybir.dt.uint16

M2 = 0x3333  # 2-bit field pairs (bits 0-1, 4-5 of each byte)
M4 = 0x0F0F  # low nibble of both bytes in a u16 lane
ALU = mybir.AluOpType

XT_BUFS = 7   # 512-token tiles: 3-tb window + 1 prefetch slot
PT_BUFS = 4
YS_BUFS = 4

# staggered (tb, og) matmul order for tb_count == 8: og k first needed at
# group index {0:0, 1:2, 2:4, 3:7} (W-prep pipeline slack), 3-tb x window
# at the start, then 2-tb-window rounds [(r,0),(r-1,1..3)] so each x-tile
# slot frees a full tb (~27us) before its reload is needed (prefetch
# margin; a pure 3-tb window gives the pool ZERO lead and starves the PE)
_MM_ORDER_8 = [
    (0, 0),
    (1, 0), (0, 1), (1, 1), (2, 0),
    (0, 2), (1, 2), (0, 3), (1, 3),
    (3, 0), (2, 1), (2, 2), (2, 3),
    (4, 0), (3, 1), (3, 2), (3, 3),
    (5, 0), (4, 1), (4, 2), (4, 3),
    (6, 0), (5, 1), (5, 2), (5, 3),
    (7, 0), (6, 1), (6, 2), (6, 3),
    (7, 1), (7, 2), (7, 3),
]

# input DMA priority order on the sync ring: ("pop", og, chunk) is a 1 MB
# 4-plane chunk, ("x", t2) is a 2 MB x tile. og0 first (gates first MM),
# then x/og interleaved to meet the staggered schedule's deadlines.
_LOAD_ORDER_8 = [
    ("pop", 0, 0), ("pop", 0, 1), ("pop", 1, 0), ("pop", 1, 1),
    ("x", 0), ("x", 1), ("x", 2),
    ("pop", 2, 0), ("pop", 2, 1), ("x", 3),
    ("pop", 3, 0), ("pop", 3, 1),
] + [("x", t) for t in range(4, 8)]

# bulk x tiles (x8-x15) are deferred to the ACT ring, issued just-in-time
# after these matmul group indices: keeps the 16 MB of tail prefetch out
# of the 0-60us window where the paired NeuronCore (same HBM stack) is
# streaming its own startup-critical bytes. ~4 group-slots (~27us) of
# lead each; the slot wait is always satisfied before issue, so the ACT
# ring never blocks.
_LATE_X_8 = {9: (8, 9), 13: (10, 11), 17: (12, 13), 21: (14, 15)}


def build_nc(tb_count: int = TOK_C // TB):
    """Build the per-core Bass program (same program on all 8 cores)."""
    t2_count = tb_count * TH
    if tb_count == TOK_C // TB:
        mm_order = _MM_ORDER_8
        load_order = _LOAD_ORDER_8
    else:
        mm_order = [(t, og) for t in range(tb_count) for og in range(OG)]
        load_order = (
            [("pop", og, pc) for og in range(OG) for pc in range(2)]
            + [("x", t) for t in range(t2_count)]
        )

    nc = bacc.Bacc(
        "TRN2",
        target_bir_lowering=False,
        debug=False,
        enable_asserts=False,
        num_devices=N_CORES,
    )

    xT = nc.dram_tensor("xT", [t2_count, P, KO, T2], BF16,
                        kind="ExternalInput")
    # 2-bit-packed population: [og, in_part, plane, ko, out128]
    nib = nc.dram_tensor("nib", [OG, P, PLANES, KO, P], U8,
                         kind="ExternalInput")
    y = nc.dram_tensor("y", [tb_count, P, OG, TB], BF16,
                       kind="ExternalOutput")

    xr = xT.ap()
    nr = nib.ap()
    yr = y.ap()

    with tile.TileContext(nc) as tc:
        with (
            tc.tile_pool(name="wsb", bufs=1) as w_pool,
            tc.tile_pool(name="pt", bufs=PT_BUFS) as pt_pool,
            tc.tile_pool(name="acc", bufs=1) as acc_pool,
            tc.tile_pool(name="tmp", bufs=1) as tmp_pool,
            tc.tile_pool(name="xt", bufs=XT_BUFS) as x_pool,
            tc.tile_pool(name="ys", bufs=YS_BUFS) as y_pool,
            tc.tile_pool(name="psum_y", bufs=6, space="PSUM") as psum_pool,
        ):
            # W^T [in_part, ko, out] bf16 -- matmul lhsT slices, SBUF-resident
            w_sb = w_pool.tile([P, KO, OUT_C], BF16, tag="wsb")

            # ---- emit load DMAs (sync ring, priority order) + W-prep (DVE)
            xt_tiles: dict = {}
            pop_chunks: dict = {}
            for item in load_order:
                if item[0] == "x":
                    t = item[1]
                    xt = x_pool.tile([P, KO, T2], BF16, tag="xt", name=f"xt{t}")
                    nc.sync.dma_start(xt[:], xr[t])
                    xt_tiles[t] = xt
                else:
                    _, og, pc = item
                    pt = pt_pool.tile([P, 4, KO, P], U8, tag="pt", name=f"pt{og}_{pc}")
                    nc.sync.dma_start(pt[:], nr[og, :, 4 * pc : 4 * pc + 4])
                    pop_chunks[(og, pc)] = pt
                    if pc == 1:
                        _emit_wprep(nc, w_sb, pop_chunks, acc_pool, tmp_pool,
                                    og)

            # ---- matmuls (PE) + drains and y stores (ACT)
            # th-granular units; for the full-size build, (1,0)-th1 moves
            # after (0,1)'s units so the x-tile-3 DMA wait (~45-53us, the
            # measured PE stall) is covered by og1 matmuls on the already
            # resident x0/x1 tiles instead of idling the PE.
            units = []
            for tb, og in mm_order:
                units.append((tb, og, 0))
                units.append((tb, og, 1))
            if tb_count == TOK_C // TB:
                units[3], units[4], units[5] = (
                    units[4], units[5], units[3]
                )
            late_x = _LATE_X_8 if tb_count == TOK_C // TB else {}
            ystages: dict = {}
            done_u: dict = {}
            for ui, (tb, og, th) in enumerate(units):
                for t in late_x.get(ui // 2, ()) if ui % 2 == 0 else ():
                    xt = x_pool.tile([P, KO, T2], BF16, tag="xt",
                                     name=f"xt{t}")
                    nc.scalar.dma_start(xt[:], xr[t])
                    xt_tiles[t] = xt
                if tb not in ystages:
                    ystages[tb] = y_pool.tile([P, OG, TB], BF16, tag="ys", name=f"ys{tb}")
                    done_u[tb] = 0
                xt = xt_tiles[tb * TH + th]
                ps = psum_pool.tile([P, T2], F32, tag="yps", name=f"ps{tb}_{og}_{th}")
                for k in range(KO):
                    nc.tensor.matmul(
                        ps[:],
                        w_sb[:, k, og * P : (og + 1) * P],
                        xt[:, k, :],
                        start=(k == 0),
                        stop=(k == KO - 1),
                    )
                nc.scalar.mul(
                    out=ystages[tb][:, og, th * T2 : (th + 1) * T2],
                    in_=ps[:],
                    mul=2.0,
                )
                done_u[tb] += 1
                if done_u[tb] == OG * TH:
                    nc.scalar.dma_start(yr[tb], ystages[tb][:])

    nc.compile()
    return nc


def _emit_wprep(nc, w_sb, pop_chunks, acc_pool, tmp_pool, og):
    """DVE: sum 8 planes (2-bit fields) -> counts -> +-1 bf16 W og-slice."""
    c0 = pop_chunks[(og, 0)][:].bitcast(U16)  # [128, 4, KO, 64] planes 0-3
    c1 = pop_chunks[(og, 1)][:].bitcast(U16)  # planes 4-7
    acc = acc_pool.tile([P, 3, KO, P // 2], U16, tag="acc", name=f"acc{og}")
    a, b, d = acc[:, 0], acc[:, 1], acc[:, 2]

    u = tmp_pool.tile([P, KO, P // 2], U16, tag="u", name=f"u{og}")
    t = tmp_pool.tile([P, KO, P // 2], U16, tag="t", name=f"t{og}")
    v = tmp_pool.tile([P, KO, P // 2], U16, tag="v", name=f"v{og}")
    cnt = tmp_pool.tile([P, KO, P // 2], U16, tag="cnt", name=f"cnt{og}")

    def _unpack(src, into_cnt):
        # (src & M2) + ((src >> 2) & M2) -> nibble fields <= 6, then
        # (v & M4) + ((v >> 4) & M4) -> byte counts <= 12. (The BIR
        # verifier forbids mixing bitwise op0 with arith op1, so mask
        # and add stay separate instructions.)
        nc.vector.tensor_scalar(
            out=u[:], in0=src, scalar1=2, scalar2=M2,
            op0=ALU.logical_shift_right, op1=ALU.bitwise_and,
        )
        nc.vector.tensor_scalar(
            out=t[:], in0=src, scalar1=M2, scalar2=None,
            op0=ALU.bitwise_and,
        )
        nc.vector.tensor_add(v[:], t[:], u[:])
        nc.vector.tensor_scalar(
            out=u[:], in0=v[:], scalar1=4, scalar2=M4,
            op0=ALU.logical_shift_right, op1=ALU.bitwise_and,
        )
        nc.vector.tensor_scalar(
            out=t[:], in0=v[:], scalar1=M4, scalar2=None,
            op0=ALU.bitwise_and,
        )
        if into_cnt:
            nc.vector.tensor_add(cnt[:], t[:], u[:])
        else:
            nc.vector.tensor_add(v[:], t[:], u[:])
            nc.vector.tensor_add(cnt[:], cnt[:], v[:])

    # three accumulators keep every 2-bit field <= 3 (no carries); u16
    # lane values stay <= 0xFFFF (exact in the fp32 ALU). All chunk-0
    # work (a + its unpack) is emitted before anything touching chunk 1
    # so a late second chunk costs nothing until ~5us into the og.
    nc.vector.tensor_add(a, c0[:, 0], c0[:, 1])
    nc.vector.tensor_add(a, a, c0[:, 2])
    _unpack(a, True)
    nc.vector.tensor_add(b, c0[:, 3], c1[:, 0])
    nc.vector.tensor_add(b, b, c1[:, 1])
    nc.vector.tensor_add(d, c1[:, 2], c1[:, 3])
    _unpack(b, False)
    _unpack(d, False)

    cnt8 = cnt[:].bitcast(U8)  # [128, KO, 128] counts in [0, 32]
    wslice = w_sb[:, :, og * P : (og + 1) * P]
    # count >= 16  <=>  swarm_sum >= 0. One op: w = (count >= 16) - 0.5
    # in {-0.5, +0.5}; the PSUM drain's activation scale=2 restores +-1
    # (exact: power-of-2 scale on an fp32 accumulation).
    nc.vector.tensor_scalar(
        out=wslice, in0=cnt8, scalar1=16, scalar2=0.5,
        op0=ALU.is_ge, op1=ALU.subtract,
    )


_NC_CACHE: dict = {}


def _get_nc(tb_count: int = TOK_C // TB):
    if tb_count not in _NC_CACHE:
        _NC_CACHE[tb_count] = build_nc(tb_count)
    return _NC_CACHE[tb_count]


def stage_x(x: np.ndarray):
    """x [b, s, in] f32 -> tiled bf16 x^T [t2_total, 128 ki, ko, 512]."""
    tokens = x.shape[0] * x.shape[1]
    xb = np.ascontiguousarray(
        x.reshape(tokens, IN_F).T
    ).astype(ml_dtypes.bfloat16)  # [in, tokens]
    t2t = tokens // T2
    # (ko ki) (t2 t) -> t2 ki ko t
    return np.ascontiguousarray(
        xb.reshape(KO, P, t2t, T2).transpose(2, 1, 0, 3)
    )


def stage_pop_quarter(pop_q: np.ndarray):
    """pop slice [512 out, in, 32] (+-1.0 f32) -> 2-bit-packed u8
    [og, ki, plane, ko, out128]; byte(plane p) = sum_j bit(s=8j+p) << 2j.
    Lossless layout-only recode (one bit per population element)."""
    b = (pop_q > 0).astype(np.uint8)  # [out 512, in 2048, s 32]
    b = b.reshape(OUT_C, IN_F, 4, PLANES)  # [out, in, j, p]
    byt = (
        b[:, :, 0, :] | (b[:, :, 1, :] << 2)
        | (b[:, :, 2, :] << 4) | (b[:, :, 3, :] << 6)
    )  # [out, in, p]
    byt = byt.reshape(OG, P, KO, P, PLANES)  # [og, o, ko, ki, p]
    return np.ascontiguousarray(byt.transpose(0, 3, 4, 2, 1))


def unstage_y(y_dev: np.ndarray):
    """y [tb, 128 r, og, TB t] bf16 -> [tok_c, out_c] f32
    (token = tb*TB + t, out = og*128 + r)."""
    tbc = y_dev.shape[0]
    return (
        y_dev.astype(np.float32)
        .transpose(0, 3, 2, 1)
        .reshape(tbc * TB, OUT_C)
    )


def prep_inputs(x: np.ndarray, population: np.ndarray):
    xT = stage_x(x)
    t2_half = TOK_C // T2
    nib_q = [
        stage_pop_quarter(population[q * OUT_C : (q + 1) * OUT_C])
        for q in range(OUT_WAYS)
    ]
    in_maps = []
    for c in range(N_CORES):
        h, q = c // OUT_WAYS, c % OUT_WAYS
        in_maps.append({
            "xT": xT[h * t2_half : (h + 1) * t2_half],
            "nib": nib_q[q],
        })
    return in_maps


def gather_y(results, batch_shape):
    y_full = np.empty((TOKENS, OUT_F), dtype=np.float32)
    for c, r in enumerate(results):
        h, q = c // OUT_WAYS, c % OUT_WAYS
        y_full[h * TOK_C : (h + 1) * TOK_C, q * OUT_C : (q + 1) * OUT_C] = (
            unstage_y(r["y"])
        )
    return y_full.reshape(*batch_shape, OUT_F)


def kernel(x: np.ndarray, population: np.ndarray):
    in_maps = prep_inputs(x, population)
    nc = _get_nc()
    res = run_bass_kernel_spmd(nc, in_maps, core_ids=list(range(N_CORES)))
    return gather_y(res.results, x.shape[:2])
